# revision 23
# baseline (speedup 1.0000x reference)
"""Trainium2 Bass kernel for nn_Block_33328946217681 (dual-stream dense
transformer: 4x [self-attn + MLP] on two streams, then one cross-attn +
MLP exchange between streams).

Sharding: 8 cores, core 2b owns x[b], core 2b+1 owns y[b] (B=4).  Each core
runs the self-block stack on its own stream; the pair (2b, 2b+1) exchanges
the *normalized* final states (bf16 AllReduce, partner = sum - own) and
runs the final cross-attention block.  Only the last loop iteration's cross
output is live in the reference, so earlier cross blocks are skipped.

Perf structure (v2):
- Attention unified for self/cross: softmax denominators ride free as a
  65th output row of the AV matmuls (V augmented with a ones column), so
  there is no Act accum_out (279ns/op on TRN2) and no denominator matmuls.
- exp fused over head pairs: one [128,1024] Act op per (pair, s-chunk)
  reading a 2-bank PSUM tile written by the two half-array score matmuls.
- LayerNorm stats (bn_stats) are emitted inline right after each residual
  add, so they overlap the remaining matmul stream; the boundary tail is
  only aggr + rsqrt + scale-apply.
- rstd computed with a single Rsqrt op (one act-table set per LN, instead
  of the Ln+Exp pair that loaded two).
- The exchange sends LN1(x_final) (bf16), channel-split into two pairwise
  AllReduces; the first half's attention pairs (heads 0-7) run while the
  second half is in flight.  Partner never re-runs LN.
- Precision: matmul operands bf16, fp32 residual stream, fp32 PSUM
  accumulation; exchange bf16 (partner = fl(fl(a+b)-a), ~1ulp noise on an
  LN'd tensor feeding the final block only).
"""

import numpy as np
import ml_dtypes

import concourse.bass as bass
import concourse.bacc as bacc
import concourse.tile as tile
from concourse import mybir
from concourse.bass_utils import run_bass_kernel_spmd

BF16 = mybir.dt.bfloat16
F32 = mybir.dt.float32
F32R = mybir.dt.float32r
AF = mybir.ActivationFunctionType
ALU = mybir.AluOpType

B, N, C = 4, 512, 1024
H, D = 16, 64
HID = 4 * C
P = 128
NT = N // P      # 4 token chunks
CCH = C // P     # 8 channel chunks
HC = HID // P    # 32 hidden chunks
PAIRS = H // 2   # 8 head pairs
EPS = 1e-5
N_CORES = 8
REPLICA_GROUPS = [[0, 1], [2, 3], [4, 5], [6, 7]]

SPLIT_EXCHANGE = True  # two channel-half collectives vs one full AllReduce

_cache = {}


def _stats_tiles(nc, pools, name):
    """Allocate bn_stats + mean/var tiles for one LN instance."""
    sb = pools["sb"]
    stats = sb.tile([P, NT, 2, 6], F32, tag="lnstats", bufs=3,
                    name=f"st_{name}")
    mv = sb.tile([P, NT, 2], F32, tag="lnmv", bufs=3, name=f"mv_{name}")
    return stats, mv


def _emit_stats(nc, stats, x_state, t, g):
    gsl = slice(g * 512, (g + 1) * 512)
    nc.vector.bn_stats(stats[:, t, g, :], x_state[:, t, gsl])


def _ln_apply(nc, pools, stats, mv, x_state, out, consts,
              aug_out=None, post_g=None):
    """aggr + rstd + scale-apply.

    out: contiguous [P,NT,C] bf16 target (feeds the PE transposes, whose
    moving operand must be single-free-dim).  aug_out: optional
    [P,NT,H,65] augmented tile for the AV matmuls; written by a second
    tensor_scalar stream emitted after the main one (off the transpose
    critical path).  post_g(g) fires after each channel half's applies
    (used to launch the exchange halves).
    """
    sb = pools["sb"]
    eps_t = pools["eps"]
    rstd = sb.tile([P, NT, 1], F32, tag="lnrstd", bufs=3, name="rstd")
    for t in range(NT):
        nc.vector.bn_aggr(mv[:, t, :], stats[:, t, :])
    # rstd = 1/sqrt(var+eps): one Act Sqrt (single table set) + DVE recip
    # (f32r ~12-bit mantissa, well above the bf16 output's 8 bits)
    nc.scalar.activation(rstd[:], mv[:, :, 1:2], AF.Sqrt, bias=eps_t[:])
    with nc.allow_low_precision(reason="LN rstd recip in f32r; output bf16"):
        nc.vector.reciprocal(rstd[:], rstd[:])
    for g in range(2):
        gsl = slice(g * 512, (g + 1) * 512)
        for t in range(NT):
            nc.vector.tensor_scalar(
                out=out[:, t, gsl], in0=x_state[:, t, gsl],
                scalar1=mv[:, t, 0:1], scalar2=rstd[:, t, :],
                op0=ALU.subtract, op1=ALU.mult)
        if post_g is not None:
            post_g(g)
    if aug_out is not None:
        for g in range(2):
            gsl = slice(g * 512, (g + 1) * 512)
            for t in range(NT):
                nc.vector.tensor_scalar(
                    out=aug_out[:, t, 8 * g:8 * (g + 1), 0:64],
                    in0=x_state[:, t, gsl],
                    scalar1=mv[:, t, 0:1], scalar2=rstd[:, t, :],
                    op0=ALU.subtract, op1=ALU.mult)


def _ln_post(nc, pools, dst, consts, gkey, bkey, aug=False):
    """Optional gain/bias application on the LN output (flags path)."""
    g_tile = consts.get(gkey)
    b_tile = consts.get(bkey)
    if g_tile is None and b_tile is None:
        return
    for t in range(NT):
        view = dst[:, t, :, 0:64] if aug else dst[:, t, :]
        if g_tile is not None:
            nc.vector.tensor_mul(view, view, g_tile[:])
        if b_tile is not None:
            nc.vector.tensor_add(view, view, b_tile[:])


def _transpose_chunks(nc, pools, src_view_fn, dst_bf, id_bf, chunks=None):
    """dst_bf[P,c,N] = transpose of token-major source, for chunk list.

    src_view_fn(t, c) -> [P,128] bf16 view of channels c*128..(c+1)*128,
    token chunk t.
    """
    ps = pools["ps"]
    chunks = range(CCH) if chunks is None else chunks
    for c in chunks:
        pst = ps.tile([P, N], BF16, tag="ps_acc", bufs=4, name=f"pstr{c}")
        for t in range(NT):
            nc.tensor.transpose(pst[:, t * P:(t + 1) * P],
                                src_view_fn(t, c), id_bf[:])
        nc.vector.tensor_copy(dst_bf[:, c, :], pst[:])


def _attention(nc, pools, qT, kv_aug, kvT, ot, consts, pre_pair=None):
    """ot[P,CCH,N] (bf16) = per-head softmax(qk/8) @ v, heads = channel dim.

    qT/kvT: [P,CCH,N] bf16 channel-major; kv_aug: [P,NT,H,65] bf16
    token-major, augmented with a ones column at [..,64] (the softmax
    denominator rides as AV output row 64, partition-aligned for DVE).  pre_pair: optional
    callable emitted before pair j's score matmuls (used to gate on the
    exchange halves in the cross block).
    """
    sb, ps = pools["sb"], pools["ps"]
    selp = consts["selp"]

    eabs = {}
    rds = {}

    def emit_scores(j):
        if pre_pair is not None:
            pre_pair(j)
        es = []
        for sc in range(NT):
            ssl = slice(sc * P, (sc + 1) * P)
            psab = ps.tile([P, 2 * N], F32, tag="psab", bufs=2,
                           name=f"psab{j}_{sc}")
            nc.tensor.matmul(psab[:, 0:N], lhsT=kvT[0:64, j, ssl],
                             rhs=qT[0:64, j, :], start=True, stop=True,
                             tile_position=(0, 0))
            nc.tensor.matmul(psab[:, N:2 * N], lhsT=kvT[64:128, j, ssl],
                             rhs=qT[64:128, j, :], start=True, stop=True,
                             tile_position=(64, 0))
            eab = sb.tile([P, 2 * N], BF16, tag="eh", bufs=5,
                          name=f"eab{j}_{sc}")
            nc.scalar.activation(eab[:], psab[:], AF.Exp, scale=0.125)
            es.append(eab)
        eabs[j] = es

    def emit_av(j):
        ha, hb = 2 * j, 2 * j + 1
        psu_a = ps.tile([P, N], F32, tag="ps_acc", bufs=4, name=f"psua{j}")
        psu_b = ps.tile([P, N], F32, tag="ps_acc", bufs=4, name=f"psub{j}")
        es = eabs.pop(j)
        for sc in range(NT):
            nc.tensor.matmul(psu_a[0:65, :],
                             lhsT=kv_aug[:, sc, ha, :],
                             rhs=es[sc][:, 0:N], start=(sc == 0),
                             stop=(sc == NT - 1), tile_position=(0, 0))
            nc.tensor.matmul(psu_b[0:65, :],
                             lhsT=kv_aug[:, sc, hb, :],
                             rhs=es[sc][:, N:2 * N], start=(sc == 0),
                             stop=(sc == NT - 1), tile_position=(0, 0))
        # U^T rows into ot; head b's 64 rows move to quadrant 2/3 (a
        # 64-partition quadrant-aligned DVE move, HW-supported)
        nc.vector.tensor_copy(ot[0:64, j, :], psu_a[0:64, :])
        nc.vector.tensor_copy(ot[64:128, j, :], psu_b[0:64, :])
        # reciprocal denominators: both heads' rows live at partition 64
        # (quadrant-aligned); they land in the two column halves of rd
        rd = sb.tile([65, 2 * N], F32R, tag="rd", bufs=2, name=f"rd{j}")
        with nc.allow_low_precision(reason="softmax denom recip in f32r"):
            nc.vector.reciprocal(rd[64:65, 0:N], psu_a[64:65, :])
            nc.vector.reciprocal(rd[64:65, N:2 * N], psu_b[64:65, :])
        rds[j] = rd

    def emit_norm(j):
        # broadcast each recip row over its head's 64 partitions: two K=1
        # matmuls ACCUMULATE into one full-width bank (f32r matmuls cannot
        # target a column-offset destination, so mask rows in selp select
        # which 64 partitions each recip lands on)
        rd = rds.pop(j)
        psc = ps.tile([P, N], F32, tag="ps_acc", bufs=4, name=f"psbc{j}")
        nc.tensor.matmul(psc[:], lhsT=selp[64:65, 0, :],
                         rhs=rd[64:65, 0:N], start=True, stop=False,
                         tile_position=(64, 0))
        nc.tensor.matmul(psc[:], lhsT=selp[64:65, 1, :],
                         rhs=rd[64:65, N:2 * N], start=False, stop=True,
                         tile_position=(64, 0))
        nc.vector.tensor_mul(ot[:, j, :], ot[:, j, :], psc[:])

    # software-pipelined emission: scores(j+1) interleaved with AV(j).
    # At the cross block's half boundary (j+1 == 4) the next scores gate
    # on the second exchange half, so AV/norm must be emitted FIRST --
    # engines are in-order and pre_pair(4)'s work would otherwise block
    # ready AV work behind the collective.
    emit_scores(0)
    for j in range(PAIRS):
        boundary = pre_pair is not None and j + 1 == 4
        if not boundary and j + 1 < PAIRS:
            emit_scores(j + 1)
        emit_av(j)
        if j >= 1:
            emit_norm(j - 1)
        if boundary:
            emit_scores(j + 1)
    emit_norm(PAIRS - 1)


def _proj_residual(nc, pools, ot, w_sb, x_state, bias_tile, stats2):
    """x_state += ot.T @ w; emits LN2 bn_stats right after each add."""
    ps = pools["ps"]
    for t in range(NT):
        for co in range(2):
            cosl = slice(co * 512, (co + 1) * 512)
            psm = ps.tile([P, 512], F32, tag="ps_acc", bufs=4,
                          name=f"pspj{t}_{co}")
            for c in range(CCH):
                nc.tensor.matmul(psm[:], lhsT=ot[:, c, t * P:(t + 1) * P],
                                 rhs=w_sb[:, c, cosl], start=(c == 0),
                                 stop=(c == CCH - 1))
            nc.vector.tensor_add(x_state[:, t, cosl], x_state[:, t, cosl],
                                 psm[:])
            if bias_tile is not None:
                nc.vector.tensor_add(x_state[:, t, cosl],
                                     x_state[:, t, cosl], bias_tile[:, cosl])
            _emit_stats(nc, stats2, x_state, t, co)


def _mlp(nc, pools, x_state, consts, stats2, mv2, stats_next, exch=None):
    """x_state += fc2(gelu(fc1(LN2(x_state)))).

    stats2/mv2: precomputed LN2 stats (from proj adds).  stats_next: if
    given, bn_stats for the NEXT block's LN1 are emitted inline after the
    fc2 residual adds.  exch: optional callable(g) fired after the fc2
    adds of channel half g (used to launch the exchange collectives).
    """
    sb, ps = pools["sb"], pools["ps"]
    x2n = sb.tile([P, NT, C], BF16, tag="n_bf", bufs=1, name="x2n")
    _ln_apply(nc, pools, stats2, mv2, x_state, x2n, consts)
    _ln_post(nc, pools, x2n, consts, "g2t", "b2t")
    x2T = sb.tile([P, CCH, N], BF16, tag="nT", bufs=2, name="x2T")
    _transpose_chunks(nc, pools,
                      lambda t, c: x2n[:, t, c * P:(c + 1) * P],
                      x2T, consts["id_bf"])

    fc1w, fc2w_dram = consts["fc1w"], consts["fc2w_dram"]
    fc1b = consts.get("fc1bt")
    hacts = []
    for ht in range(HC):
        psh = ps.tile([P, N], F32, tag="ps_acc", bufs=4, name=f"psh{ht}")
        for c in range(CCH):
            nc.tensor.matmul(psh[:], lhsT=fc1w[:, c, ht * P:(ht + 1) * P],
                             rhs=x2T[:, c, :], start=(c == 0),
                             stop=(c == CCH - 1))
        hact = sb.tile([P, N], BF16, tag="hact", bufs=32, name=f"hact{ht}")
        if fc1b is not None:
            nc.scalar.activation(hact[:], psh[:], AF.Gelu,
                                 bias=fc1b[:, ht:ht + 1])
        else:
            nc.scalar.activation(hact[:], psh[:], AF.Gelu)
        hacts.append(hact)

    fc2b = consts.get("fc2bt")
    for co in range(2):
        cosl = slice(co * 512, (co + 1) * 512)
        psms = [ps.tile([P, 512], F32, tag="ps_acc", bufs=4,
                        name=f"psm2_{co}_{t}") for t in range(NT)]
        for hc in range(HC):
            wt = sb.tile([P, 512], BF16, tag="fc2w", bufs=3,
                         name=f"f2w{co}_{hc}")
            nc.sync.dma_start(wt[:], fc2w_dram[hc * P:(hc + 1) * P, cosl])
            for t in range(NT):
                nc.tensor.matmul(psms[t][:],
                                 lhsT=hacts[hc][:, t * P:(t + 1) * P],
                                 rhs=wt[:], start=(hc == 0),
                                 stop=(hc == HC - 1))
        for t in range(NT):
            nc.vector.tensor_add(x_state[:, t, cosl], x_state[:, t, cosl],
                                 psms[t][:])
            if fc2b is not None:
                nc.vector.tensor_add(x_state[:, t, cosl],
                                     x_state[:, t, cosl], fc2b[:, cosl])
            if stats_next is not None:
                _emit_stats(nc, stats_next, x_state, t, co)
        if exch is not None:
            exch(co)


def _self_block(nc, pools, x_state, consts, stats1, mv1, stats_next):
    """One self-attention transformer block; stats1 precomputed."""
    sb = pools["sb"]
    kv_aug = pools["kv_aug"]
    xn = sb.tile([P, NT, C], BF16, tag="n_bf", bufs=1, name="xn")
    _ln_apply(nc, pools, stats1, mv1, x_state, xn, consts, aug_out=kv_aug)
    _ln_post(nc, pools, xn, consts, "g1t", "b1t")
    _ln_post(nc, pools, kv_aug, consts, "g1t", "b1t", aug=True)
    xnT = sb.tile([P, CCH, N], BF16, tag="nT", bufs=2, name="xnT")
    _transpose_chunks(nc, pools,
                      lambda t, c: xn[:, t, c * P:(c + 1) * P],
                      xnT, consts["id_bf"])

    ot = sb.tile([P, CCH, N], BF16, tag="ot", bufs=1, name="ot")
    _attention(nc, pools, xnT, kv_aug, xnT, ot, consts)
    stats2, mv2 = _stats_tiles(nc, pools, "ln2")
    _proj_residual(nc, pools, ot, consts["projw"], x_state,
                   consts.get("projbt"), stats2)
    _mlp(nc, pools, x_state, consts, stats2, mv2, stats_next)


def _build(n_self, flags):
    """flags: dict of bools: g1,b1,g2,b2,projb,fc1b,fc2b nontrivial."""
    nc = bacc.Bacc("TRN2", target_bir_lowering=False, debug=False,
                   num_devices=N_CORES)

    own_d = nc.dram_tensor("own", [P, NT, C], F32, kind="ExternalInput").ap()
    projw_d = nc.dram_tensor("projw", [P, CCH, C], BF16,
                             kind="ExternalInput").ap()
    fc1w_d = nc.dram_tensor("fc1w", [P, CCH, HID], BF16,
                            kind="ExternalInput").ap()
    fc2w_d = nc.dram_tensor("fc2w", [HID, C], BF16, kind="ExternalInput").ap()
    idbf_d = nc.dram_tensor("id_bf", [P, P], BF16, kind="ExternalInput").ap()
    selp_d = nc.dram_tensor("selp", [65, 2, P], F32R, kind="ExternalInput").ap()
    extra_d = {}
    for nm, shape in (("g1", [C]), ("b1", [C]), ("g2", [C]), ("b2", [C]),
                      ("projb", [C]), ("fc2b", [C])):
        if flags[nm]:
            extra_d[nm] = nc.dram_tensor(nm, shape, F32,
                                         kind="ExternalInput").ap()
    if flags["fc1b"]:
        extra_d["fc1b"] = nc.dram_tensor("fc1b", [P, HC], F32,
                                         kind="ExternalInput").ap()
    out_d = nc.dram_tensor("out", [P, NT, C], F32, kind="ExternalOutput").ap()

    with tile.TileContext(nc) as tc:
        with tc.tile_pool(name="sb", bufs=1) as sb, \
             tc.tile_pool(name="ps", bufs=1, space="PSUM") as ps, \
             tc.tile_pool(name="dram", bufs=1, space="DRAM") as dram:
            pools = {"sb": sb, "ps": ps, "dram": dram}
            eps_t = sb.tile([P, 1], F32, tag="eps", name="eps_t")
            nc.vector.memset(eps_t[:], EPS)
            pools["eps"] = eps_t

            # persistent state + resident weights + constants.  DMA order
            # matters: the SP queue is serial, and the first block's LN and
            # transposes need id_bf + x_state -- queue the small constants
            # and x_state BEFORE the 10MB of weights, or the PE idles ~35us
            # at startup waiting for id_bf behind fc1w.
            id_bf = sb.tile([P, P], BF16, tag="id_bf", name="id_bf")
            nc.sync.dma_start(id_bf[:], idbf_d)
            selp = sb.tile([65, 2, P], F32R, tag="selp", name="selp")
            nc.sync.dma_start(selp[:], selp_d)
            x_state = sb.tile([P, NT, C], F32, tag="x_state", name="x_state")
            nc.sync.dma_start(x_state[:], own_d)
            projw = sb.tile([P, CCH, C], BF16, tag="projw", name="projw")
            nc.sync.dma_start(projw[:], projw_d)
            fc1w = sb.tile([P, CCH, HID], BF16, tag="fc1w", name="fc1w")
            nc.sync.dma_start(fc1w[:], fc1w_d)

            # augmented kv layouts: col 64 = ones for even heads, col 65 =
            # ones for odd heads (denominator rows of the AV matmuls)
            kv_aug = sb.tile([P, NT, H, 65], BF16, tag="kv_aug",
                             name="kv_aug")
            nc.vector.memset(kv_aug[:, :, :, 64:65], 1.0)
            pools["kv_aug"] = kv_aug
            # the cross block reuses kv_aug for the partner (the self
            # blocks' contents are dead by then)
            pn_aug = kv_aug

            consts = {"id_bf": id_bf, "selp": selp, "projw": projw,
                      "fc1w": fc1w, "fc2w_dram": fc2w_d}
            # optional gain/bias tiles
            for nm, key in (("g1", "g1t"), ("b1", "b1t"), ("g2", "g2t"),
                            ("b2", "b2t"), ("projb", "projbt"),
                            ("fc2b", "fc2bt")):
                if flags[nm]:
                    t_ = sb.tile([P, C], F32, tag=nm, name=nm + "t")
                    nc.sync.dma_start(t_[:],
                                      extra_d[nm].to_broadcast((P, C)))
                    consts[key] = t_
            if flags["fc1b"]:
                t_ = sb.tile([P, HC], F32, tag="fc1b", name="fc1bt")
                nc.sync.dma_start(t_[:], extra_d["fc1b"])
                consts["fc1bt"] = t_

            # prologue LN1 stats for the first block
            stats1, mv1 = _stats_tiles(nc, pools, "ln1a")
            for t in range(NT):
                for g in range(2):
                    _emit_stats(nc, stats1, x_state, t, g)

            # exchange buffers (dram).  Layout [2, P, NT, 512]: each
            # channel half is contiguous (collective APs must be), and the
            # whole buffer is contiguous too for the single-collective mode.
            snd = dram.tile([2, P, NT, 512], BF16, name="snd")
            rcv = dram.tile([2, P, NT, 512], BF16, name="rcv")
            xn5 = sb.tile([P, NT, C], BF16, tag="xn5", name="xn5")

            def exch(g):
                gsl = slice(g * 512, (g + 1) * 512)
                nc.sync.dma_start(snd[g], xn5[:, :, gsl])
                if SPLIT_EXCHANGE:
                    nc.gpsimd.collective_compute(
                        "AllReduce", ALU.add, replica_groups=REPLICA_GROUPS,
                        ins=[snd[g].opt()], outs=[rcv[g].opt()])
                elif g == 1:
                    # one collective over both halves (this runtime pays a
                    # large per-collective cost, so one beats two)
                    nc.gpsimd.collective_compute(
                        "AllReduce", ALU.add, replica_groups=REPLICA_GROUPS,
                        ins=[snd[:].opt()], outs=[rcv[:].opt()])

            for k in range(n_self):
                stats_next, mv_next = _stats_tiles(nc, pools, f"ln1_{k}")
                _self_block(nc, pools, x_state, consts, stats1, mv1,
                            stats_next)
                stats1, mv1 = stats_next, mv_next

            # ---- cross block ----
            # own LN1 -> xn5 (contiguous: exchange payload + Q source);
            # each channel half's collective fires as soon as its scale-
            # applies are emitted
            has_gb1 = "g1t" in consts or "b1t" in consts
            _ln_apply(nc, pools, stats1, mv1, x_state, xn5, consts,
                      post_g=None if has_gb1 else exch)
            if has_gb1:
                _ln_post(nc, pools, xn5, consts, "g1t", "b1t")
                exch(0)
                exch(1)
            xnT = sb.tile([P, CCH, N], BF16, tag="nT", bufs=2, name="xnT5")
            _transpose_chunks(nc, pools,
                              lambda t, c: xn5[:, t, c * P:(c + 1) * P],
                              xnT, consts["id_bf"])

            # partner = allreduced - own, written straight into the
            # augmented kv layout, per channel half as halves arrive
            rcv_sb = sb.tile([P, NT, C], BF16, tag="rcv_sb", name="rcv_sb")
            pn = sb.tile([P, NT, C], BF16, tag="n_bf", bufs=1, name="pn")
            kvT = sb.tile([P, CCH, N], BF16, tag="nT", bufs=2, name="pnT")
            ready_halves = set()

            def pre_pair(j):
                g = 0 if j < 4 else 1
                if g in ready_halves:
                    return
                ready_halves.add(g)
                gsl = slice(g * 512, (g + 1) * 512)
                nc.sync.dma_start(rcv_sb[:, :, gsl], rcv[g])
                for t in range(NT):
                    nc.vector.tensor_sub(pn[:, t, gsl],
                                         rcv_sb[:, t, gsl], xn5[:, t, gsl])
                _transpose_chunks(nc, pools,
                                  lambda t, c: pn[:, t, c * P:(c + 1) * P],
                                  kvT, consts["id_bf"],
                                  chunks=range(4 * g, 4 * (g + 1)))
                # augmented copy for the AV matmuls (off the transpose
                # critical path)
                for t in range(NT):
                    nc.vector.tensor_sub(
                        pn_aug[:, t, 8 * g:8 * (g + 1), 0:64],
                        rcv_sb[:, t, gsl], xn5[:, t, gsl])

            ot = sb.tile([P, CCH, N], BF16, tag="ot", bufs=1, name="otx")
            _attention(nc, pools, xnT, pn_aug, kvT, ot, consts,
                       pre_pair=pre_pair)
            stats2, mv2 = _stats_tiles(nc, pools, "ln2x")
            _proj_residual(nc, pools, ot, consts["projw"], x_state,
                           consts.get("projbt"), stats2)
            _mlp(nc, pools, x_state, consts, stats2, mv2, None)

            nc.sync.dma_start(out_d, x_state[:])
    nc.compile()
    return nc


def _get_nc(n_self, flags):
    key = (n_self, tuple(sorted(flags.items())))
    if key not in _cache:
        _cache[key] = _build(n_self, flags)
    return _cache[key]


def _nontrivial(a, val=0.0):
    return not np.allclose(np.asarray(a, np.float32), val, atol=0.0, rtol=0.0)


def kernel(**inputs):
    x = np.ascontiguousarray(np.asarray(inputs["x"], np.float32))
    y = np.ascontiguousarray(np.asarray(inputs["y"], np.float32))
    n1g, n1b = inputs["norm1_g"], inputs["norm1_b"]
    n2g, n2b = inputs["norm2_g"], inputs["norm2_b"]
    proj_w, proj_b = inputs["proj_w"], inputs["proj_b"]
    fc1_w, fc1_b = inputs["fc1_w"], inputs["fc1_b"]
    fc2_w, fc2_b = inputs["fc2_w"], inputs["fc2_b"]
    is_selfatt = int(np.asarray(inputs["is_selfatt"]))

    flags = {
        "g1": _nontrivial(n1g, 1.0), "b1": _nontrivial(n1b),
        "g2": _nontrivial(n2g, 1.0), "b2": _nontrivial(n2b),
        "projb": _nontrivial(proj_b), "fc1b": _nontrivial(fc1_b),
        "fc2b": _nontrivial(fc2_b),
    }
    n_self = 4 if is_selfatt else 0
    nc = _get_nc(n_self, flags)

    bf = ml_dtypes.bfloat16
    projw_h = np.ascontiguousarray(
        np.asarray(proj_w, np.float32).reshape(CCH, P, C).transpose(1, 0, 2)
    ).astype(bf)
    fc1w_h = np.ascontiguousarray(
        np.asarray(fc1_w, np.float32).reshape(CCH, P, HID).transpose(1, 0, 2)
    ).astype(bf)
    fc2w_h = np.ascontiguousarray(np.asarray(fc2_w, np.float32)).astype(bf)
    id_h = np.eye(P, dtype=np.float32)
    selp_h = np.zeros((65, 2, P), np.float32)
    selp_h[64, 0, 0:64] = 1.0
    selp_h[64, 1, 64:128] = 1.0

    base = {
        "projw": projw_h, "fc1w": fc1w_h, "fc2w": fc2w_h,
        "id_bf": id_h.astype(bf), "selp": selp_h,
    }
    for nm, arr in (("g1", n1g), ("b1", n1b), ("g2", n2g), ("b2", n2b),
                    ("projb", proj_b), ("fc2b", fc2_b)):
        if flags[nm]:
            base[nm] = np.ascontiguousarray(np.asarray(arr, np.float32))
    if flags["fc1b"]:
        base["fc1b"] = np.ascontiguousarray(
            np.asarray(fc1_b, np.float32).reshape(HC, P).T)

    in_maps = []
    for core in range(N_CORES):
        bidx = core // 2
        own = x[bidx] if core % 2 == 0 else y[bidx]
        own_dev = np.ascontiguousarray(
            own.reshape(NT, P, C).transpose(1, 0, 2))
        m = dict(base)
        m["own"] = own_dev
        in_maps.append(m)

    res = run_bass_kernel_spmd(nc, in_maps, core_ids=list(range(N_CORES)))

    def unpack(core):
        o = np.asarray(res.results[core]["out"], np.float32)
        return o.transpose(1, 0, 2).reshape(N, C)

    x1 = np.stack([unpack(2 * b) for b in range(B)])
    y1 = np.stack([unpack(2 * b + 1) for b in range(B)])
    return (x1, y1)


# revision 24
# speedup vs baseline: 1.0127x; 1.0127x over previous
"""Trainium2 Bass kernel for nn_Block_33328946217681 (dual-stream dense
transformer: 4x [self-attn + MLP] on two streams, then one cross-attn +
MLP exchange between streams).

Sharding: 8 cores, core 2b owns x[b], core 2b+1 owns y[b] (B=4).  Each core
runs the self-block stack on its own stream; the pair (2b, 2b+1) exchanges
the *normalized* final states (bf16 AllReduce, partner = sum - own) and
runs the final cross-attention block.  Only the last loop iteration's cross
output is live in the reference, so earlier cross blocks are skipped.

Perf structure (v2):
- Attention unified for self/cross: softmax denominators ride free as a
  65th output row of the AV matmuls (V augmented with a ones column), so
  there is no Act accum_out (279ns/op on TRN2) and no denominator matmuls.
- exp fused over head pairs: one [128,1024] Act op per (pair, s-chunk)
  reading a 2-bank PSUM tile written by the two half-array score matmuls.
- LayerNorm stats (bn_stats) are emitted inline right after each residual
  add, so they overlap the remaining matmul stream; the boundary tail is
  only aggr + rsqrt + scale-apply.
- rstd computed with a single Rsqrt op (one act-table set per LN, instead
  of the Ln+Exp pair that loaded two).
- The exchange sends LN1(x_final) (bf16), channel-split into two pairwise
  AllReduces; the first half's attention pairs (heads 0-7) run while the
  second half is in flight.  Partner never re-runs LN.
- Precision: matmul operands bf16, fp32 residual stream, fp32 PSUM
  accumulation; exchange bf16 (partner = fl(fl(a+b)-a), ~1ulp noise on an
  LN'd tensor feeding the final block only).
"""

import numpy as np
import ml_dtypes

import concourse.bass as bass
import concourse.bacc as bacc
import concourse.tile as tile
from concourse import mybir
from concourse.bass_utils import run_bass_kernel_spmd

BF16 = mybir.dt.bfloat16
F32 = mybir.dt.float32
F32R = mybir.dt.float32r
AF = mybir.ActivationFunctionType
ALU = mybir.AluOpType

B, N, C = 4, 512, 1024
H, D = 16, 64
HID = 4 * C
P = 128
NT = N // P      # 4 token chunks
CCH = C // P     # 8 channel chunks
HC = HID // P    # 32 hidden chunks
PAIRS = H // 2   # 8 head pairs
EPS = 1e-5
N_CORES = 8
REPLICA_GROUPS = [[0, 1], [2, 3], [4, 5], [6, 7]]

SPLIT_EXCHANGE = True  # two channel-half collectives vs one full AllReduce

_cache = {}


def _stats_tiles(nc, pools, name):
    """Allocate bn_stats + mean/var tiles for one LN instance."""
    sb = pools["sb"]
    stats = sb.tile([P, NT, 2, 6], F32, tag="lnstats", bufs=3,
                    name=f"st_{name}")
    mv = sb.tile([P, NT, 2], F32, tag="lnmv", bufs=3, name=f"mv_{name}")
    return stats, mv


def _emit_stats(nc, stats, x_state, t, g):
    gsl = slice(g * 512, (g + 1) * 512)
    nc.vector.bn_stats(stats[:, t, g, :], x_state[:, t, gsl])


def _ln_apply(nc, pools, stats, mv, x_state, out, consts,
              aug_out=None, post_g=None):
    """aggr + rstd + scale-apply.

    out: contiguous [P,NT,C] bf16 target (feeds the PE transposes, whose
    moving operand must be single-free-dim).  aug_out: optional
    [P,NT,H,65] augmented tile for the AV matmuls; written by a second
    tensor_scalar stream emitted after the main one (off the transpose
    critical path).  post_g(g) fires after each channel half's applies
    (used to launch the exchange halves).
    """
    sb = pools["sb"]
    eps_t = pools["eps"]
    rstd = sb.tile([P, NT, 1], F32, tag="lnrstd", bufs=3, name="rstd")

    def rstd_chain(t0, t1):
        for t in range(t0, t1):
            nc.vector.bn_aggr(mv[:, t, :], stats[:, t, :])
        # rstd = 1/sqrt(var+eps): Act Sqrt + DVE recip (f32r ~12-bit
        # mantissa, well above the bf16 output's 8 bits)
        nc.scalar.activation(rstd[:, t0:t1, :], mv[:, t0:t1, 1:2],
                             AF.Sqrt, bias=eps_t[:])
        with nc.allow_low_precision(reason="LN rstd recip; output bf16"):
            nc.vector.reciprocal(rstd[:, t0:t1, :], rstd[:, t0:t1, :])

    def apply_one(t, g):
        gsl = slice(g * 512, (g + 1) * 512)
        nc.vector.tensor_scalar(
            out=out[:, t, gsl], in0=x_state[:, t, gsl],
            scalar1=mv[:, t, 0:1], scalar2=rstd[:, t, :],
            op0=ALU.subtract, op1=ALU.mult)

    if post_g is None:
        # engines are in-order: t3's stats land last (its residual add is
        # the final one), so run tokens 0-2 through aggr/sqrt/recip/apply
        # first and keep only t3's own chain in the boundary tail
        rstd_chain(0, NT - 1)
        for t in range(NT - 1):
            for g in range(2):
                apply_one(t, g)
        rstd_chain(NT - 1, NT)
        for g in range(2):
            apply_one(NT - 1, g)
    else:
        rstd_chain(0, NT)
        for g in range(2):
            for t in range(NT):
                apply_one(t, g)
            post_g(g)
    if aug_out is not None:
        for g in range(2):
            gsl = slice(g * 512, (g + 1) * 512)
            for t in range(NT):
                nc.vector.tensor_scalar(
                    out=aug_out[:, t, 8 * g:8 * (g + 1), 0:64],
                    in0=x_state[:, t, gsl],
                    scalar1=mv[:, t, 0:1], scalar2=rstd[:, t, :],
                    op0=ALU.subtract, op1=ALU.mult)


def _ln_post(nc, pools, dst, consts, gkey, bkey, aug=False):
    """Optional gain/bias application on the LN output (flags path)."""
    g_tile = consts.get(gkey)
    b_tile = consts.get(bkey)
    if g_tile is None and b_tile is None:
        return
    for t in range(NT):
        view = dst[:, t, :, 0:64] if aug else dst[:, t, :]
        if g_tile is not None:
            nc.vector.tensor_mul(view, view, g_tile[:])
        if b_tile is not None:
            nc.vector.tensor_add(view, view, b_tile[:])


def _transpose_chunks(nc, pools, src_view_fn, dst_bf, id_bf, chunks=None):
    """dst_bf[P,c,N] = transpose of token-major source, for chunk list.

    src_view_fn(t, c) -> [P,128] bf16 view of channels c*128..(c+1)*128,
    token chunk t.
    """
    ps = pools["ps"]
    chunks = range(CCH) if chunks is None else chunks
    for c in chunks:
        pst = ps.tile([P, N], BF16, tag="ps_acc", bufs=4, name=f"pstr{c}")
        for t in range(NT):
            nc.tensor.transpose(pst[:, t * P:(t + 1) * P],
                                src_view_fn(t, c), id_bf[:])
        nc.vector.tensor_copy(dst_bf[:, c, :], pst[:])


def _attention(nc, pools, qT, kv_aug, kvT, ot, consts, pre_pair=None):
    """ot[P,CCH,N] (bf16) = per-head softmax(qk/8) @ v, heads = channel dim.

    qT/kvT: [P,CCH,N] bf16 channel-major; kv_aug: [P,NT,H,65] bf16
    token-major, augmented with a ones column at [..,64] (the softmax
    denominator rides as AV output row 64, partition-aligned for DVE).  pre_pair: optional
    callable emitted before pair j's score matmuls (used to gate on the
    exchange halves in the cross block).
    """
    sb, ps = pools["sb"], pools["ps"]
    selp = consts["selp"]

    eabs = {}
    rds = {}

    def emit_scores(j):
        if pre_pair is not None:
            pre_pair(j)
        es = []
        for sc in range(NT):
            ssl = slice(sc * P, (sc + 1) * P)
            psab = ps.tile([P, 2 * N], F32, tag="psab", bufs=2,
                           name=f"psab{j}_{sc}")
            nc.tensor.matmul(psab[:, 0:N], lhsT=kvT[0:64, j, ssl],
                             rhs=qT[0:64, j, :], start=True, stop=True,
                             tile_position=(0, 0))
            nc.tensor.matmul(psab[:, N:2 * N], lhsT=kvT[64:128, j, ssl],
                             rhs=qT[64:128, j, :], start=True, stop=True,
                             tile_position=(64, 0))
            eab = sb.tile([P, 2 * N], BF16, tag="eh", bufs=5,
                          name=f"eab{j}_{sc}")
            nc.scalar.activation(eab[:], psab[:], AF.Exp, scale=0.125)
            es.append(eab)
        eabs[j] = es

    def emit_av(j):
        ha, hb = 2 * j, 2 * j + 1
        psu_a = ps.tile([P, N], F32, tag="ps_acc", bufs=4, name=f"psua{j}")
        psu_b = ps.tile([P, N], F32, tag="ps_acc", bufs=4, name=f"psub{j}")
        es = eabs.pop(j)
        for sc in range(NT):
            nc.tensor.matmul(psu_a[0:65, :],
                             lhsT=kv_aug[:, sc, ha, :],
                             rhs=es[sc][:, 0:N], start=(sc == 0),
                             stop=(sc == NT - 1), tile_position=(0, 0))
            nc.tensor.matmul(psu_b[0:65, :],
                             lhsT=kv_aug[:, sc, hb, :],
                             rhs=es[sc][:, N:2 * N], start=(sc == 0),
                             stop=(sc == NT - 1), tile_position=(0, 0))
        # U^T rows into ot; head b's 64 rows move to quadrant 2/3 (a
        # 64-partition quadrant-aligned DVE move, HW-supported)
        nc.vector.tensor_copy(ot[0:64, j, :], psu_a[0:64, :])
        nc.vector.tensor_copy(ot[64:128, j, :], psu_b[0:64, :])
        # reciprocal denominators: both heads' rows live at partition 64
        # (quadrant-aligned); they land in the two column halves of rd
        rd = sb.tile([65, 2 * N], F32R, tag="rd", bufs=2, name=f"rd{j}")
        with nc.allow_low_precision(reason="softmax denom recip in f32r"):
            nc.vector.reciprocal(rd[64:65, 0:N], psu_a[64:65, :])
            nc.vector.reciprocal(rd[64:65, N:2 * N], psu_b[64:65, :])
        rds[j] = rd

    def emit_norm(j):
        # broadcast each recip row over its head's 64 partitions: two K=1
        # matmuls ACCUMULATE into one full-width bank (f32r matmuls cannot
        # target a column-offset destination, so mask rows in selp select
        # which 64 partitions each recip lands on)
        rd = rds.pop(j)
        psc = ps.tile([P, N], F32, tag="ps_acc", bufs=4, name=f"psbc{j}")
        nc.tensor.matmul(psc[:], lhsT=selp[64:65, 0, :],
                         rhs=rd[64:65, 0:N], start=True, stop=False,
                         tile_position=(64, 0))
        nc.tensor.matmul(psc[:], lhsT=selp[64:65, 1, :],
                         rhs=rd[64:65, N:2 * N], start=False, stop=True,
                         tile_position=(64, 0))
        nc.vector.tensor_mul(ot[:, j, :], ot[:, j, :], psc[:])

    # software-pipelined emission: scores(j+1) interleaved with AV(j).
    # At the cross block's half boundary (j+1 == 4) the next scores gate
    # on the second exchange half, so AV/norm must be emitted FIRST --
    # engines are in-order and pre_pair(4)'s work would otherwise block
    # ready AV work behind the collective.
    emit_scores(0)
    for j in range(PAIRS):
        boundary = pre_pair is not None and j + 1 == 4
        if not boundary and j + 1 < PAIRS:
            emit_scores(j + 1)
        emit_av(j)
        if j >= 1:
            emit_norm(j - 1)
        if boundary:
            emit_scores(j + 1)
    emit_norm(PAIRS - 1)


def _proj_residual(nc, pools, ot, w_sb, x_state, bias_tile, stats2):
    """x_state += ot.T @ w; emits LN2 bn_stats right after each add."""
    ps = pools["ps"]
    for t in range(NT):
        for co in range(2):
            cosl = slice(co * 512, (co + 1) * 512)
            psm = ps.tile([P, 512], F32, tag="ps_acc", bufs=4,
                          name=f"pspj{t}_{co}")
            for c in range(CCH):
                nc.tensor.matmul(psm[:], lhsT=ot[:, c, t * P:(t + 1) * P],
                                 rhs=w_sb[:, c, cosl], start=(c == 0),
                                 stop=(c == CCH - 1))
            nc.vector.tensor_add(x_state[:, t, cosl], x_state[:, t, cosl],
                                 psm[:])
            if bias_tile is not None:
                nc.vector.tensor_add(x_state[:, t, cosl],
                                     x_state[:, t, cosl], bias_tile[:, cosl])
            _emit_stats(nc, stats2, x_state, t, co)


def _mlp(nc, pools, x_state, consts, stats2, mv2, stats_next, exch=None):
    """x_state += fc2(gelu(fc1(LN2(x_state)))).

    stats2/mv2: precomputed LN2 stats (from proj adds).  stats_next: if
    given, bn_stats for the NEXT block's LN1 are emitted inline after the
    fc2 residual adds.  exch: optional callable(g) fired after the fc2
    adds of channel half g (used to launch the exchange collectives).
    """
    sb, ps = pools["sb"], pools["ps"]
    x2n = sb.tile([P, NT, C], BF16, tag="n_bf", bufs=1, name="x2n")
    _ln_apply(nc, pools, stats2, mv2, x_state, x2n, consts)
    _ln_post(nc, pools, x2n, consts, "g2t", "b2t")
    x2T = sb.tile([P, CCH, N], BF16, tag="nT", bufs=2, name="x2T")
    _transpose_chunks(nc, pools,
                      lambda t, c: x2n[:, t, c * P:(c + 1) * P],
                      x2T, consts["id_bf"])

    fc1w, fc2w_dram = consts["fc1w"], consts["fc2w_dram"]
    fc1b = consts.get("fc1bt")
    hacts = []
    for ht in range(HC):
        psh = ps.tile([P, N], F32, tag="ps_acc", bufs=4, name=f"psh{ht}")
        for c in range(CCH):
            nc.tensor.matmul(psh[:], lhsT=fc1w[:, c, ht * P:(ht + 1) * P],
                             rhs=x2T[:, c, :], start=(c == 0),
                             stop=(c == CCH - 1))
        hact = sb.tile([P, N], BF16, tag="hact", bufs=32, name=f"hact{ht}")
        if fc1b is not None:
            nc.scalar.activation(hact[:], psh[:], AF.Gelu,
                                 bias=fc1b[:, ht:ht + 1])
        else:
            nc.scalar.activation(hact[:], psh[:], AF.Gelu)
        hacts.append(hact)

    fc2b = consts.get("fc2bt")
    for co in range(2):
        cosl = slice(co * 512, (co + 1) * 512)
        psms = [ps.tile([P, 512], F32, tag="ps_acc", bufs=4,
                        name=f"psm2_{co}_{t}") for t in range(NT)]
        for hc in range(HC):
            wt = sb.tile([P, 512], BF16, tag="fc2w", bufs=3,
                         name=f"f2w{co}_{hc}")
            nc.sync.dma_start(wt[:], fc2w_dram[hc * P:(hc + 1) * P, cosl])
            for t in range(NT):
                nc.tensor.matmul(psms[t][:],
                                 lhsT=hacts[hc][:, t * P:(t + 1) * P],
                                 rhs=wt[:], start=(hc == 0),
                                 stop=(hc == HC - 1))
        for t in range(NT):
            nc.vector.tensor_add(x_state[:, t, cosl], x_state[:, t, cosl],
                                 psms[t][:])
            if fc2b is not None:
                nc.vector.tensor_add(x_state[:, t, cosl],
                                     x_state[:, t, cosl], fc2b[:, cosl])
            if stats_next is not None:
                _emit_stats(nc, stats_next, x_state, t, co)
        if exch is not None:
            exch(co)


def _self_block(nc, pools, x_state, consts, stats1, mv1, stats_next):
    """One self-attention transformer block; stats1 precomputed."""
    sb = pools["sb"]
    kv_aug = pools["kv_aug"]
    xn = sb.tile([P, NT, C], BF16, tag="n_bf", bufs=1, name="xn")
    _ln_apply(nc, pools, stats1, mv1, x_state, xn, consts, aug_out=kv_aug)
    _ln_post(nc, pools, xn, consts, "g1t", "b1t")
    _ln_post(nc, pools, kv_aug, consts, "g1t", "b1t", aug=True)
    xnT = sb.tile([P, CCH, N], BF16, tag="nT", bufs=2, name="xnT")
    _transpose_chunks(nc, pools,
                      lambda t, c: xn[:, t, c * P:(c + 1) * P],
                      xnT, consts["id_bf"])

    ot = sb.tile([P, CCH, N], BF16, tag="ot", bufs=1, name="ot")
    _attention(nc, pools, xnT, kv_aug, xnT, ot, consts)
    stats2, mv2 = _stats_tiles(nc, pools, "ln2")
    _proj_residual(nc, pools, ot, consts["projw"], x_state,
                   consts.get("projbt"), stats2)
    _mlp(nc, pools, x_state, consts, stats2, mv2, stats_next)


def _build(n_self, flags):
    """flags: dict of bools: g1,b1,g2,b2,projb,fc1b,fc2b nontrivial."""
    nc = bacc.Bacc("TRN2", target_bir_lowering=False, debug=False,
                   num_devices=N_CORES)

    own_d = nc.dram_tensor("own", [P, NT, C], F32, kind="ExternalInput").ap()
    projw_d = nc.dram_tensor("projw", [P, CCH, C], BF16,
                             kind="ExternalInput").ap()
    fc1w_d = nc.dram_tensor("fc1w", [P, CCH, HID], BF16,
                            kind="ExternalInput").ap()
    fc2w_d = nc.dram_tensor("fc2w", [HID, C], BF16, kind="ExternalInput").ap()
    idbf_d = nc.dram_tensor("id_bf", [P, P], BF16, kind="ExternalInput").ap()
    selp_d = nc.dram_tensor("selp", [65, 2, P], F32R, kind="ExternalInput").ap()
    extra_d = {}
    for nm, shape in (("g1", [C]), ("b1", [C]), ("g2", [C]), ("b2", [C]),
                      ("projb", [C]), ("fc2b", [C])):
        if flags[nm]:
            extra_d[nm] = nc.dram_tensor(nm, shape, F32,
                                         kind="ExternalInput").ap()
    if flags["fc1b"]:
        extra_d["fc1b"] = nc.dram_tensor("fc1b", [P, HC], F32,
                                         kind="ExternalInput").ap()
    out_d = nc.dram_tensor("out", [P, NT, C], F32, kind="ExternalOutput").ap()

    with tile.TileContext(nc) as tc:
        with tc.tile_pool(name="sb", bufs=1) as sb, \
             tc.tile_pool(name="ps", bufs=1, space="PSUM") as ps, \
             tc.tile_pool(name="dram", bufs=1, space="DRAM") as dram:
            pools = {"sb": sb, "ps": ps, "dram": dram}
            eps_t = sb.tile([P, 1], F32, tag="eps", name="eps_t")
            nc.vector.memset(eps_t[:], EPS)
            pools["eps"] = eps_t

            # persistent state + resident weights + constants.  DMA order
            # matters: the SP queue is serial, and the first block's LN and
            # transposes need id_bf + x_state -- queue the small constants
            # and x_state BEFORE the 10MB of weights, or the PE idles ~35us
            # at startup waiting for id_bf behind fc1w.
            id_bf = sb.tile([P, P], BF16, tag="id_bf", name="id_bf")
            nc.sync.dma_start(id_bf[:], idbf_d)
            selp = sb.tile([65, 2, P], F32R, tag="selp", name="selp")
            nc.sync.dma_start(selp[:], selp_d)
            x_state = sb.tile([P, NT, C], F32, tag="x_state", name="x_state")
            nc.sync.dma_start(x_state[:], own_d)
            projw = sb.tile([P, CCH, C], BF16, tag="projw", name="projw")
            nc.sync.dma_start(projw[:], projw_d)
            fc1w = sb.tile([P, CCH, HID], BF16, tag="fc1w", name="fc1w")
            nc.sync.dma_start(fc1w[:], fc1w_d)

            # augmented kv layouts: col 64 = ones for even heads, col 65 =
            # ones for odd heads (denominator rows of the AV matmuls)
            kv_aug = sb.tile([P, NT, H, 65], BF16, tag="kv_aug",
                             name="kv_aug")
            nc.vector.memset(kv_aug[:, :, :, 64:65], 1.0)
            pools["kv_aug"] = kv_aug
            # the cross block reuses kv_aug for the partner (the self
            # blocks' contents are dead by then)
            pn_aug = kv_aug

            consts = {"id_bf": id_bf, "selp": selp, "projw": projw,
                      "fc1w": fc1w, "fc2w_dram": fc2w_d}
            # optional gain/bias tiles
            for nm, key in (("g1", "g1t"), ("b1", "b1t"), ("g2", "g2t"),
                            ("b2", "b2t"), ("projb", "projbt"),
                            ("fc2b", "fc2bt")):
                if flags[nm]:
                    t_ = sb.tile([P, C], F32, tag=nm, name=nm + "t")
                    nc.sync.dma_start(t_[:],
                                      extra_d[nm].to_broadcast((P, C)))
                    consts[key] = t_
            if flags["fc1b"]:
                t_ = sb.tile([P, HC], F32, tag="fc1b", name="fc1bt")
                nc.sync.dma_start(t_[:], extra_d["fc1b"])
                consts["fc1bt"] = t_

            # prologue LN1 stats for the first block
            stats1, mv1 = _stats_tiles(nc, pools, "ln1a")
            for t in range(NT):
                for g in range(2):
                    _emit_stats(nc, stats1, x_state, t, g)

            # exchange buffers (dram).  Layout [2, P, NT, 512]: each
            # channel half is contiguous (collective APs must be), and the
            # whole buffer is contiguous too for the single-collective mode.
            snd = dram.tile([2, P, NT, 512], BF16, name="snd")
            rcv = dram.tile([2, P, NT, 512], BF16, name="rcv")
            xn5 = sb.tile([P, NT, C], BF16, tag="xn5", name="xn5")

            def exch(g):
                gsl = slice(g * 512, (g + 1) * 512)
                nc.sync.dma_start(snd[g], xn5[:, :, gsl])
                if SPLIT_EXCHANGE:
                    nc.gpsimd.collective_compute(
                        "AllReduce", ALU.add, replica_groups=REPLICA_GROUPS,
                        ins=[snd[g].opt()], outs=[rcv[g].opt()])
                elif g == 1:
                    # one collective over both halves (this runtime pays a
                    # large per-collective cost, so one beats two)
                    nc.gpsimd.collective_compute(
                        "AllReduce", ALU.add, replica_groups=REPLICA_GROUPS,
                        ins=[snd[:].opt()], outs=[rcv[:].opt()])

            for k in range(n_self):
                stats_next, mv_next = _stats_tiles(nc, pools, f"ln1_{k}")
                _self_block(nc, pools, x_state, consts, stats1, mv1,
                            stats_next)
                stats1, mv1 = stats_next, mv_next

            # ---- cross block ----
            # own LN1 -> xn5 (contiguous: exchange payload + Q source);
            # each channel half's collective fires as soon as its scale-
            # applies are emitted
            has_gb1 = "g1t" in consts or "b1t" in consts
            _ln_apply(nc, pools, stats1, mv1, x_state, xn5, consts,
                      post_g=None if has_gb1 else exch)
            if has_gb1:
                _ln_post(nc, pools, xn5, consts, "g1t", "b1t")
                exch(0)
                exch(1)
            xnT = sb.tile([P, CCH, N], BF16, tag="nT", bufs=2, name="xnT5")
            _transpose_chunks(nc, pools,
                              lambda t, c: xn5[:, t, c * P:(c + 1) * P],
                              xnT, consts["id_bf"])

            # partner = allreduced - own, written straight into the
            # augmented kv layout, per channel half as halves arrive
            rcv_sb = sb.tile([P, NT, C], BF16, tag="rcv_sb", name="rcv_sb")
            pn = sb.tile([P, NT, C], BF16, tag="n_bf", bufs=1, name="pn")
            kvT = sb.tile([P, CCH, N], BF16, tag="nT", bufs=2, name="pnT")
            ready_halves = set()

            def pre_pair(j):
                g = 0 if j < 4 else 1
                if g in ready_halves:
                    return
                ready_halves.add(g)
                gsl = slice(g * 512, (g + 1) * 512)
                nc.sync.dma_start(rcv_sb[:, :, gsl], rcv[g])
                for t in range(NT):
                    nc.vector.tensor_sub(pn[:, t, gsl],
                                         rcv_sb[:, t, gsl], xn5[:, t, gsl])
                _transpose_chunks(nc, pools,
                                  lambda t, c: pn[:, t, c * P:(c + 1) * P],
                                  kvT, consts["id_bf"],
                                  chunks=range(4 * g, 4 * (g + 1)))
                # augmented copy for the AV matmuls (off the transpose
                # critical path)
                for t in range(NT):
                    nc.vector.tensor_sub(
                        pn_aug[:, t, 8 * g:8 * (g + 1), 0:64],
                        rcv_sb[:, t, gsl], xn5[:, t, gsl])

            ot = sb.tile([P, CCH, N], BF16, tag="ot", bufs=1, name="otx")
            _attention(nc, pools, xnT, pn_aug, kvT, ot, consts,
                       pre_pair=pre_pair)
            stats2, mv2 = _stats_tiles(nc, pools, "ln2x")
            _proj_residual(nc, pools, ot, consts["projw"], x_state,
                           consts.get("projbt"), stats2)
            _mlp(nc, pools, x_state, consts, stats2, mv2, None)

            nc.sync.dma_start(out_d, x_state[:])
    nc.compile()
    return nc


def _get_nc(n_self, flags):
    key = (n_self, tuple(sorted(flags.items())))
    if key not in _cache:
        _cache[key] = _build(n_self, flags)
    return _cache[key]


def _nontrivial(a, val=0.0):
    return not np.allclose(np.asarray(a, np.float32), val, atol=0.0, rtol=0.0)


def kernel(**inputs):
    x = np.ascontiguousarray(np.asarray(inputs["x"], np.float32))
    y = np.ascontiguousarray(np.asarray(inputs["y"], np.float32))
    n1g, n1b = inputs["norm1_g"], inputs["norm1_b"]
    n2g, n2b = inputs["norm2_g"], inputs["norm2_b"]
    proj_w, proj_b = inputs["proj_w"], inputs["proj_b"]
    fc1_w, fc1_b = inputs["fc1_w"], inputs["fc1_b"]
    fc2_w, fc2_b = inputs["fc2_w"], inputs["fc2_b"]
    is_selfatt = int(np.asarray(inputs["is_selfatt"]))

    flags = {
        "g1": _nontrivial(n1g, 1.0), "b1": _nontrivial(n1b),
        "g2": _nontrivial(n2g, 1.0), "b2": _nontrivial(n2b),
        "projb": _nontrivial(proj_b), "fc1b": _nontrivial(fc1_b),
        "fc2b": _nontrivial(fc2_b),
    }
    n_self = 4 if is_selfatt else 0
    nc = _get_nc(n_self, flags)

    bf = ml_dtypes.bfloat16
    projw_h = np.ascontiguousarray(
        np.asarray(proj_w, np.float32).reshape(CCH, P, C).transpose(1, 0, 2)
    ).astype(bf)
    fc1w_h = np.ascontiguousarray(
        np.asarray(fc1_w, np.float32).reshape(CCH, P, HID).transpose(1, 0, 2)
    ).astype(bf)
    fc2w_h = np.ascontiguousarray(np.asarray(fc2_w, np.float32)).astype(bf)
    id_h = np.eye(P, dtype=np.float32)
    selp_h = np.zeros((65, 2, P), np.float32)
    selp_h[64, 0, 0:64] = 1.0
    selp_h[64, 1, 64:128] = 1.0

    base = {
        "projw": projw_h, "fc1w": fc1w_h, "fc2w": fc2w_h,
        "id_bf": id_h.astype(bf), "selp": selp_h,
    }
    for nm, arr in (("g1", n1g), ("b1", n1b), ("g2", n2g), ("b2", n2b),
                    ("projb", proj_b), ("fc2b", fc2_b)):
        if flags[nm]:
            base[nm] = np.ascontiguousarray(np.asarray(arr, np.float32))
    if flags["fc1b"]:
        base["fc1b"] = np.ascontiguousarray(
            np.asarray(fc1_b, np.float32).reshape(HC, P).T)

    in_maps = []
    for core in range(N_CORES):
        bidx = core // 2
        own = x[bidx] if core % 2 == 0 else y[bidx]
        own_dev = np.ascontiguousarray(
            own.reshape(NT, P, C).transpose(1, 0, 2))
        m = dict(base)
        m["own"] = own_dev
        in_maps.append(m)

    res = run_bass_kernel_spmd(nc, in_maps, core_ids=list(range(N_CORES)))

    def unpack(core):
        o = np.asarray(res.results[core]["out"], np.float32)
        return o.transpose(1, 0, 2).reshape(N, C)

    x1 = np.stack([unpack(2 * b) for b in range(B)])
    y1 = np.stack([unpack(2 * b + 1) for b in range(B)])
    return (x1, y1)


# revision 28
# speedup vs baseline: 1.0188x; 1.0061x over previous
"""Trainium2 Bass kernel for nn_Block_33328946217681 (dual-stream dense
transformer: 4x [self-attn + MLP] on two streams, then one cross-attn +
MLP exchange between streams).

Sharding: 8 cores, core 2b owns x[b], core 2b+1 owns y[b] (B=4).  Each core
runs the self-block stack on its own stream; the pair (2b, 2b+1) exchanges
the *normalized* final states (bf16 AllReduce, partner = sum - own) and
runs the final cross-attention block.  Only the last loop iteration's cross
output is live in the reference, so earlier cross blocks are skipped.

Perf structure (v2):
- Attention unified for self/cross: softmax denominators ride free as a
  65th output row of the AV matmuls (V augmented with a ones column), so
  there is no Act accum_out (279ns/op on TRN2) and no denominator matmuls.
- exp fused over head pairs: one [128,1024] Act op per (pair, s-chunk)
  reading a 2-bank PSUM tile written by the two half-array score matmuls.
- LayerNorm stats (bn_stats) are emitted inline right after each residual
  add, so they overlap the remaining matmul stream; the boundary tail is
  only aggr + rsqrt + scale-apply.
- rstd computed with a single Rsqrt op (one act-table set per LN, instead
  of the Ln+Exp pair that loaded two).
- The exchange sends LN1(x_final) (bf16), channel-split into two pairwise
  AllReduces; the first half's attention pairs (heads 0-7) run while the
  second half is in flight.  Partner never re-runs LN.
- Precision: matmul operands bf16, fp32 residual stream, fp32 PSUM
  accumulation; exchange bf16 (partner = fl(fl(a+b)-a), ~1ulp noise on an
  LN'd tensor feeding the final block only).
"""

import numpy as np
import ml_dtypes

import concourse.bass as bass
import concourse.bacc as bacc
import concourse.tile as tile
from concourse import mybir
from concourse.bass_utils import run_bass_kernel_spmd

BF16 = mybir.dt.bfloat16
F32 = mybir.dt.float32
F32R = mybir.dt.float32r
AF = mybir.ActivationFunctionType
ALU = mybir.AluOpType

B, N, C = 4, 512, 1024
H, D = 16, 64
HID = 4 * C
P = 128
NT = N // P      # 4 token chunks
CCH = C // P     # 8 channel chunks
HC = HID // P    # 32 hidden chunks
PAIRS = H // 2   # 8 head pairs
EPS = 1e-5
N_CORES = 8
REPLICA_GROUPS = [[0, 1], [2, 3], [4, 5], [6, 7]]

SPLIT_EXCHANGE = True  # two channel-half collectives vs one full AllReduce

_cache = {}


def _stats_tiles(nc, pools, name):
    """Allocate bn_stats + mean/var tiles for one LN instance."""
    sb = pools["sb"]
    stats = sb.tile([P, NT, 2, 6], F32, tag="lnstats", bufs=3,
                    name=f"st_{name}")
    mv = sb.tile([P, NT, 2], F32, tag="lnmv", bufs=3, name=f"mv_{name}")
    return stats, mv


def _emit_stats(nc, stats, x_state, t, g):
    gsl = slice(g * 512, (g + 1) * 512)
    nc.vector.bn_stats(stats[:, t, g, :], x_state[:, t, gsl])


def _ln_apply(nc, pools, stats, mv, x_state, out, consts,
              aug_out=None, post_g=None):
    """aggr + rstd + scale-apply.

    out: contiguous [P,NT,C] bf16 target (feeds the PE transposes, whose
    moving operand must be single-free-dim).  aug_out: optional
    [P,NT,H,65] augmented tile for the AV matmuls; written by a second
    tensor_scalar stream emitted after the main one (off the transpose
    critical path).  post_g(g) fires after each channel half's applies
    (used to launch the exchange halves).
    """
    sb = pools["sb"]
    eps_t = pools["eps"]
    rstd = sb.tile([P, NT, 1], F32, tag="lnrstd", bufs=3, name="rstd")

    def rstd_chain(t0, t1):
        for t in range(t0, t1):
            nc.vector.bn_aggr(mv[:, t, :], stats[:, t, :])
        # rstd = 1/sqrt(var+eps): Act Sqrt + DVE recip (f32r ~12-bit
        # mantissa, well above the bf16 output's 8 bits)
        nc.scalar.activation(rstd[:, t0:t1, :], mv[:, t0:t1, 1:2],
                             AF.Sqrt, bias=eps_t[:])
        with nc.allow_low_precision(reason="LN rstd recip; output bf16"):
            nc.vector.reciprocal(rstd[:, t0:t1, :], rstd[:, t0:t1, :])

    def apply_one(t, g):
        gsl = slice(g * 512, (g + 1) * 512)
        nc.vector.tensor_scalar(
            out=out[:, t, gsl], in0=x_state[:, t, gsl],
            scalar1=mv[:, t, 0:1], scalar2=rstd[:, t, :],
            op0=ALU.subtract, op1=ALU.mult)

    if post_g is None:
        # engines are in-order: t3's stats land last (its residual add is
        # the final one), so run tokens 0-2 through aggr/sqrt/recip/apply
        # first and keep only t3's own chain in the boundary tail
        rstd_chain(0, NT - 1)
        for t in range(NT - 1):
            for g in range(2):
                apply_one(t, g)
        rstd_chain(NT - 1, NT)
        for g in range(2):
            apply_one(NT - 1, g)
    else:
        rstd_chain(0, NT)
        for g in range(2):
            for t in range(NT):
                apply_one(t, g)
            post_g(g)
    if aug_out is not None:
        _ln_aug(nc, mv, rstd, x_state, aug_out)
    return rstd


def _ln_aug(nc, mv, rstd, x_state, aug_out):
    for g in range(2):
        gsl = slice(g * 512, (g + 1) * 512)
        for t in range(NT):
            nc.vector.tensor_scalar(
                out=aug_out[:, t, 8 * g:8 * (g + 1), 0:64],
                in0=x_state[:, t, gsl],
                scalar1=mv[:, t, 0:1], scalar2=rstd[:, t, :],
                op0=ALU.subtract, op1=ALU.mult)


def _ln_post(nc, pools, dst, consts, gkey, bkey, aug=False):
    """Optional gain/bias application on the LN output (flags path)."""
    g_tile = consts.get(gkey)
    b_tile = consts.get(bkey)
    if g_tile is None and b_tile is None:
        return
    for t in range(NT):
        view = dst[:, t, :, 0:64] if aug else dst[:, t, :]
        if g_tile is not None:
            nc.vector.tensor_mul(view, view, g_tile[:])
        if b_tile is not None:
            nc.vector.tensor_add(view, view, b_tile[:])


def _transpose_chunks(nc, pools, src_view_fn, dst_bf, id_bf, chunks=None):
    """dst_bf[P,c,N] = transpose of token-major source, for chunk list.

    src_view_fn(t, c) -> [P,128] bf16 view of channels c*128..(c+1)*128,
    token chunk t.
    """
    ps = pools["ps"]
    chunks = range(CCH) if chunks is None else chunks
    for c in chunks:
        pst = ps.tile([P, N], BF16, tag="ps_acc", bufs=4, name=f"pstr{c}")
        for t in range(NT):
            nc.tensor.transpose(pst[:, t * P:(t + 1) * P],
                                src_view_fn(t, c), id_bf[:])
        nc.vector.tensor_copy(dst_bf[:, c, :], pst[:])


def _attention(nc, pools, qT, kv_aug, kvT, ot, consts, pre_pair=None):
    """ot[P,CCH,N] (bf16) = per-head softmax(qk/8) @ v, heads = channel dim.

    qT/kvT: [P,CCH,N] bf16 channel-major; kv_aug: [P,NT,H,65] bf16
    token-major, augmented with a ones column at [..,64] (the softmax
    denominator rides as AV output row 64, partition-aligned for DVE).  pre_pair: optional
    callable emitted before pair j's score matmuls (used to gate on the
    exchange halves in the cross block).
    """
    sb, ps = pools["sb"], pools["ps"]
    selp = consts["selp"]

    eabs = {}
    rds = {}

    def emit_scores(j):
        if pre_pair is not None:
            pre_pair(j)
        es = []
        for sc in range(NT):
            ssl = slice(sc * P, (sc + 1) * P)
            psab = ps.tile([P, 2 * N], F32, tag="psab", bufs=2,
                           name=f"psab{j}_{sc}")
            nc.tensor.matmul(psab[:, 0:N], lhsT=kvT[0:64, j, ssl],
                             rhs=qT[0:64, j, :], start=True, stop=True,
                             tile_position=(0, 0))
            nc.tensor.matmul(psab[:, N:2 * N], lhsT=kvT[64:128, j, ssl],
                             rhs=qT[64:128, j, :], start=True, stop=True,
                             tile_position=(64, 0))
            eab = sb.tile([P, 2 * N], BF16, tag="eh", bufs=5,
                          name=f"eab{j}_{sc}")
            nc.scalar.activation(eab[:], psab[:], AF.Exp, scale=0.125)
            es.append(eab)
        eabs[j] = es

    def emit_av(j):
        ha, hb = 2 * j, 2 * j + 1
        psu_a = ps.tile([P, N], F32, tag="ps_acc", bufs=4, name=f"psua{j}")
        psu_b = ps.tile([P, N], F32, tag="ps_acc", bufs=4, name=f"psub{j}")
        es = eabs.pop(j)
        for sc in range(NT):
            nc.tensor.matmul(psu_a[0:65, :],
                             lhsT=kv_aug[:, sc, ha, :],
                             rhs=es[sc][:, 0:N], start=(sc == 0),
                             stop=(sc == NT - 1), tile_position=(0, 0))
            nc.tensor.matmul(psu_b[0:65, :],
                             lhsT=kv_aug[:, sc, hb, :],
                             rhs=es[sc][:, N:2 * N], start=(sc == 0),
                             stop=(sc == NT - 1), tile_position=(0, 0))
        # U^T rows into ot; head b's 64 rows move to quadrant 2/3 (a
        # 64-partition quadrant-aligned DVE move, HW-supported)
        nc.vector.tensor_copy(ot[0:64, j, :], psu_a[0:64, :])
        nc.vector.tensor_copy(ot[64:128, j, :], psu_b[0:64, :])
        # reciprocal denominators: both heads' rows live at partition 64
        # (quadrant-aligned); they land in the two column halves of rd
        rd = sb.tile([65, 2 * N], F32R, tag="rd", bufs=2, name=f"rd{j}")
        with nc.allow_low_precision(reason="softmax denom recip in f32r"):
            nc.vector.reciprocal(rd[64:65, 0:N], psu_a[64:65, :])
            nc.vector.reciprocal(rd[64:65, N:2 * N], psu_b[64:65, :])
        rds[j] = rd

    def emit_norm(j):
        # broadcast each recip row over its head's 64 partitions: two K=1
        # matmuls ACCUMULATE into one full-width bank (f32r matmuls cannot
        # target a column-offset destination, so mask rows in selp select
        # which 64 partitions each recip lands on)
        rd = rds.pop(j)
        psc = ps.tile([P, N], F32, tag="ps_acc", bufs=4, name=f"psbc{j}")
        nc.tensor.matmul(psc[:], lhsT=selp[64:65, 0, :],
                         rhs=rd[64:65, 0:N], start=True, stop=False,
                         tile_position=(64, 0))
        nc.tensor.matmul(psc[:], lhsT=selp[64:65, 1, :],
                         rhs=rd[64:65, N:2 * N], start=False, stop=True,
                         tile_position=(64, 0))
        nc.vector.tensor_mul(ot[:, j, :], ot[:, j, :], psc[:])

    # software-pipelined emission: scores(j+1) interleaved with AV(j).
    # At the cross block's half boundary (j+1 == 4) the next scores gate
    # on the second exchange half, so AV/norm must be emitted FIRST --
    # engines are in-order and pre_pair(4)'s work would otherwise block
    # ready AV work behind the collective.
    emit_scores(0)
    for j in range(PAIRS):
        boundary = pre_pair is not None and j + 1 == 4
        if not boundary and j + 1 < PAIRS:
            emit_scores(j + 1)
        emit_av(j)
        if j >= 1:
            emit_norm(j - 1)
        if boundary:
            emit_scores(j + 1)
    emit_norm(PAIRS - 1)


def _proj_residual(nc, pools, ot, w_sb, x_state, bias_tile, stats2):
    """x_state += ot.T @ w; emits LN2 bn_stats right after each add."""
    ps = pools["ps"]
    for t in range(NT):
        for co in range(2):
            cosl = slice(co * 512, (co + 1) * 512)
            psm = ps.tile([P, 512], F32, tag="ps_acc", bufs=4,
                          name=f"pspj{t}_{co}")
            for c in range(CCH):
                nc.tensor.matmul(psm[:], lhsT=ot[:, c, t * P:(t + 1) * P],
                                 rhs=w_sb[:, c, cosl], start=(c == 0),
                                 stop=(c == CCH - 1))
            nc.vector.tensor_add(x_state[:, t, cosl], x_state[:, t, cosl],
                                 psm[:])
            if bias_tile is not None:
                nc.vector.tensor_add(x_state[:, t, cosl],
                                     x_state[:, t, cosl], bias_tile[:, cosl])
            _emit_stats(nc, stats2, x_state, t, co)


def _mlp(nc, pools, x_state, consts, stats2, mv2, stats_next, exch=None):
    """x_state += fc2(gelu(fc1(LN2(x_state)))).

    stats2/mv2: precomputed LN2 stats (from proj adds).  stats_next: if
    given, bn_stats for the NEXT block's LN1 are emitted inline after the
    fc2 residual adds.  exch: optional callable(g) fired after the fc2
    adds of channel half g (used to launch the exchange collectives).
    """
    sb, ps = pools["sb"], pools["ps"]
    x2n = sb.tile([P, NT, C], BF16, tag="n_bf", bufs=1, name="x2n")
    _ln_apply(nc, pools, stats2, mv2, x_state, x2n, consts)
    _ln_post(nc, pools, x2n, consts, "g2t", "b2t")
    x2T = sb.tile([P, CCH, N], BF16, tag="nT", bufs=2, name="x2T")
    _transpose_chunks(nc, pools,
                      lambda t, c: x2n[:, t, c * P:(c + 1) * P],
                      x2T, consts["id_bf"])

    fc1w, fc2w_dram = consts["fc1w"], consts["fc2w_dram"]
    fc1b = consts.get("fc1bt")
    hacts = []
    for ht in range(HC):
        psh = ps.tile([P, N], F32, tag="ps_acc", bufs=4, name=f"psh{ht}")
        for c in range(CCH):
            nc.tensor.matmul(psh[:], lhsT=fc1w[:, c, ht * P:(ht + 1) * P],
                             rhs=x2T[:, c, :], start=(c == 0),
                             stop=(c == CCH - 1))
        hact = sb.tile([P, N], BF16, tag="hact", bufs=32, name=f"hact{ht}")
        if fc1b is not None:
            nc.scalar.activation(hact[:], psh[:], AF.Gelu,
                                 bias=fc1b[:, ht:ht + 1])
        else:
            nc.scalar.activation(hact[:], psh[:], AF.Gelu)
        hacts.append(hact)

    fc2b = consts.get("fc2bt")
    for co in range(2):
        cosl = slice(co * 512, (co + 1) * 512)
        psms = [ps.tile([P, 512], F32, tag="ps_acc", bufs=4,
                        name=f"psm2_{co}_{t}") for t in range(NT)]
        for hc in range(HC):
            wt = sb.tile([P, 512], BF16, tag="fc2w", bufs=3,
                         name=f"f2w{co}_{hc}")
            nc.sync.dma_start(wt[:], fc2w_dram[hc * P:(hc + 1) * P, cosl])
            for t in range(NT):
                nc.tensor.matmul(psms[t][:],
                                 lhsT=hacts[hc][:, t * P:(t + 1) * P],
                                 rhs=wt[:], start=(hc == 0),
                                 stop=(hc == HC - 1))
        for t in range(NT):
            nc.vector.tensor_add(x_state[:, t, cosl], x_state[:, t, cosl],
                                 psms[t][:])
            if fc2b is not None:
                nc.vector.tensor_add(x_state[:, t, cosl],
                                     x_state[:, t, cosl], fc2b[:, cosl])
            if stats_next is not None:
                _emit_stats(nc, stats_next, x_state, t, co)
        if exch is not None:
            exch(co)


def _self_block(nc, pools, x_state, consts, stats1, mv1, stats_next):
    """One self-attention transformer block; stats1 precomputed."""
    sb = pools["sb"]
    kv_aug = pools["kv_aug"]
    xn = sb.tile([P, NT, C], BF16, tag="n_bf", bufs=1, name="xn")
    rstd = _ln_apply(nc, pools, stats1, mv1, x_state, xn, consts)
    _ln_post(nc, pools, xn, consts, "g1t", "b1t")
    xnT = sb.tile([P, CCH, N], BF16, tag="nT", bufs=2, name="xnT")
    _transpose_chunks(nc, pools,
                      lambda t, c: xn[:, t, c * P:(c + 1) * P],
                      xnT, consts["id_bf"])
    # augmented copy for the AV matmuls, emitted after the transpose
    # evacs so they don't queue behind it on the in-order DVE
    _ln_aug(nc, mv1, rstd, x_state, kv_aug)
    _ln_post(nc, pools, kv_aug, consts, "g1t", "b1t", aug=True)

    ot = sb.tile([P, CCH, N], BF16, tag="ot", bufs=1, name="ot")
    _attention(nc, pools, xnT, kv_aug, xnT, ot, consts)
    stats2, mv2 = _stats_tiles(nc, pools, "ln2")
    _proj_residual(nc, pools, ot, consts["projw"], x_state,
                   consts.get("projbt"), stats2)
    _mlp(nc, pools, x_state, consts, stats2, mv2, stats_next)


def _build(n_self, flags):
    """flags: dict of bools: g1,b1,g2,b2,projb,fc1b,fc2b nontrivial."""
    nc = bacc.Bacc("TRN2", target_bir_lowering=False, debug=False,
                   num_devices=N_CORES)

    own_d = nc.dram_tensor("own", [P, NT, C], F32, kind="ExternalInput").ap()
    projw_d = nc.dram_tensor("projw", [P, CCH, C], BF16,
                             kind="ExternalInput").ap()
    fc1w_d = nc.dram_tensor("fc1w", [P, CCH, HID], BF16,
                            kind="ExternalInput").ap()
    fc2w_d = nc.dram_tensor("fc2w", [HID, C], BF16, kind="ExternalInput").ap()
    idbf_d = nc.dram_tensor("id_bf", [P, P], BF16, kind="ExternalInput").ap()
    selp_d = nc.dram_tensor("selp", [65, 2, P], F32R, kind="ExternalInput").ap()
    extra_d = {}
    for nm, shape in (("g1", [C]), ("b1", [C]), ("g2", [C]), ("b2", [C]),
                      ("projb", [C]), ("fc2b", [C])):
        if flags[nm]:
            extra_d[nm] = nc.dram_tensor(nm, shape, F32,
                                         kind="ExternalInput").ap()
    if flags["fc1b"]:
        extra_d["fc1b"] = nc.dram_tensor("fc1b", [P, HC], F32,
                                         kind="ExternalInput").ap()
    out_d = nc.dram_tensor("out", [P, NT, C], F32, kind="ExternalOutput").ap()

    with tile.TileContext(nc) as tc:
        with tc.tile_pool(name="sb", bufs=1) as sb, \
             tc.tile_pool(name="ps", bufs=1, space="PSUM") as ps, \
             tc.tile_pool(name="dram", bufs=1, space="DRAM") as dram:
            pools = {"sb": sb, "ps": ps, "dram": dram}
            eps_t = sb.tile([P, 1], F32, tag="eps", name="eps_t")
            nc.vector.memset(eps_t[:], EPS)
            pools["eps"] = eps_t

            # persistent state + resident weights + constants.  DMA order
            # matters: the SP queue is serial, and the first block's LN and
            # transposes need id_bf + x_state -- queue the small constants
            # and x_state BEFORE the 10MB of weights, or the PE idles ~35us
            # at startup waiting for id_bf behind fc1w.
            id_bf = sb.tile([P, P], BF16, tag="id_bf", name="id_bf")
            nc.sync.dma_start(id_bf[:], idbf_d)
            selp = sb.tile([65, 2, P], F32R, tag="selp", name="selp")
            nc.sync.dma_start(selp[:], selp_d)
            x_state = sb.tile([P, NT, C], F32, tag="x_state", name="x_state")
            nc.sync.dma_start(x_state[:], own_d)
            projw = sb.tile([P, CCH, C], BF16, tag="projw", name="projw")
            nc.sync.dma_start(projw[:], projw_d)
            fc1w = sb.tile([P, CCH, HID], BF16, tag="fc1w", name="fc1w")
            nc.sync.dma_start(fc1w[:], fc1w_d)

            # augmented kv layouts: col 64 = ones for even heads, col 65 =
            # ones for odd heads (denominator rows of the AV matmuls)
            kv_aug = sb.tile([P, NT, H, 65], BF16, tag="kv_aug",
                             name="kv_aug")
            nc.vector.memset(kv_aug[:, :, :, 64:65], 1.0)
            pools["kv_aug"] = kv_aug
            # the cross block reuses kv_aug for the partner (the self
            # blocks' contents are dead by then)
            pn_aug = kv_aug

            consts = {"id_bf": id_bf, "selp": selp, "projw": projw,
                      "fc1w": fc1w, "fc2w_dram": fc2w_d}
            # optional gain/bias tiles
            for nm, key in (("g1", "g1t"), ("b1", "b1t"), ("g2", "g2t"),
                            ("b2", "b2t"), ("projb", "projbt"),
                            ("fc2b", "fc2bt")):
                if flags[nm]:
                    t_ = sb.tile([P, C], F32, tag=nm, name=nm + "t")
                    nc.sync.dma_start(t_[:],
                                      extra_d[nm].to_broadcast((P, C)))
                    consts[key] = t_
            if flags["fc1b"]:
                t_ = sb.tile([P, HC], F32, tag="fc1b", name="fc1bt")
                nc.sync.dma_start(t_[:], extra_d["fc1b"])
                consts["fc1bt"] = t_

            # prologue LN1 stats for the first block
            stats1, mv1 = _stats_tiles(nc, pools, "ln1a")
            for t in range(NT):
                for g in range(2):
                    _emit_stats(nc, stats1, x_state, t, g)

            # exchange buffers (dram).  Layout [2, P, NT, 512]: each
            # channel half is contiguous (collective APs must be), and the
            # whole buffer is contiguous too for the single-collective mode.
            snd = dram.tile([2, P, NT, 512], BF16, name="snd")
            rcv = dram.tile([2, P, NT, 512], BF16, name="rcv")
            xn5 = sb.tile([P, NT, C], BF16, tag="xn5", name="xn5")

            def exch(g):
                gsl = slice(g * 512, (g + 1) * 512)
                nc.sync.dma_start(snd[g], xn5[:, :, gsl])
                if SPLIT_EXCHANGE:
                    nc.gpsimd.collective_compute(
                        "AllReduce", ALU.add, replica_groups=REPLICA_GROUPS,
                        ins=[snd[g].opt()], outs=[rcv[g].opt()])
                elif g == 1:
                    # one collective over both halves (this runtime pays a
                    # large per-collective cost, so one beats two)
                    nc.gpsimd.collective_compute(
                        "AllReduce", ALU.add, replica_groups=REPLICA_GROUPS,
                        ins=[snd[:].opt()], outs=[rcv[:].opt()])

            for k in range(n_self):
                stats_next, mv_next = _stats_tiles(nc, pools, f"ln1_{k}")
                _self_block(nc, pools, x_state, consts, stats1, mv1,
                            stats_next)
                stats1, mv1 = stats_next, mv_next

            # ---- cross block ----
            # own LN1 -> xn5 (contiguous: exchange payload + Q source);
            # each channel half's collective fires as soon as its scale-
            # applies are emitted
            has_gb1 = "g1t" in consts or "b1t" in consts
            _ln_apply(nc, pools, stats1, mv1, x_state, xn5, consts,
                      post_g=None if has_gb1 else exch)
            if has_gb1:
                _ln_post(nc, pools, xn5, consts, "g1t", "b1t")
                exch(0)
                exch(1)
            xnT = sb.tile([P, CCH, N], BF16, tag="nT", bufs=2, name="xnT5")
            _transpose_chunks(nc, pools,
                              lambda t, c: xn5[:, t, c * P:(c + 1) * P],
                              xnT, consts["id_bf"])

            # partner = allreduced - own, written straight into the
            # augmented kv layout, per channel half as halves arrive
            rcv_sb = sb.tile([P, NT, C], BF16, tag="rcv_sb", name="rcv_sb")
            pn = sb.tile([P, NT, C], BF16, tag="n_bf", bufs=1, name="pn")
            kvT = sb.tile([P, CCH, N], BF16, tag="nT", bufs=2, name="pnT")
            ready_halves = set()

            def pre_pair(j):
                g = 0 if j < 4 else 1
                if g in ready_halves:
                    return
                ready_halves.add(g)
                gsl = slice(g * 512, (g + 1) * 512)
                nc.sync.dma_start(rcv_sb[:, :, gsl], rcv[g])
                for t in range(NT):
                    nc.vector.tensor_sub(pn[:, t, gsl],
                                         rcv_sb[:, t, gsl], xn5[:, t, gsl])
                _transpose_chunks(nc, pools,
                                  lambda t, c: pn[:, t, c * P:(c + 1) * P],
                                  kvT, consts["id_bf"],
                                  chunks=range(4 * g, 4 * (g + 1)))
                # augmented copy for the AV matmuls (off the transpose
                # critical path)
                for t in range(NT):
                    nc.vector.tensor_sub(
                        pn_aug[:, t, 8 * g:8 * (g + 1), 0:64],
                        rcv_sb[:, t, gsl], xn5[:, t, gsl])

            ot = sb.tile([P, CCH, N], BF16, tag="ot", bufs=1, name="otx")
            _attention(nc, pools, xnT, pn_aug, kvT, ot, consts,
                       pre_pair=pre_pair)
            stats2, mv2 = _stats_tiles(nc, pools, "ln2x")
            _proj_residual(nc, pools, ot, consts["projw"], x_state,
                           consts.get("projbt"), stats2)
            _mlp(nc, pools, x_state, consts, stats2, mv2, None)

            nc.sync.dma_start(out_d, x_state[:])
    nc.compile()
    return nc


def _get_nc(n_self, flags):
    key = (n_self, tuple(sorted(flags.items())))
    if key not in _cache:
        _cache[key] = _build(n_self, flags)
    return _cache[key]


def _nontrivial(a, val=0.0):
    return not np.allclose(np.asarray(a, np.float32), val, atol=0.0, rtol=0.0)


def kernel(**inputs):
    x = np.ascontiguousarray(np.asarray(inputs["x"], np.float32))
    y = np.ascontiguousarray(np.asarray(inputs["y"], np.float32))
    n1g, n1b = inputs["norm1_g"], inputs["norm1_b"]
    n2g, n2b = inputs["norm2_g"], inputs["norm2_b"]
    proj_w, proj_b = inputs["proj_w"], inputs["proj_b"]
    fc1_w, fc1_b = inputs["fc1_w"], inputs["fc1_b"]
    fc2_w, fc2_b = inputs["fc2_w"], inputs["fc2_b"]
    is_selfatt = int(np.asarray(inputs["is_selfatt"]))

    flags = {
        "g1": _nontrivial(n1g, 1.0), "b1": _nontrivial(n1b),
        "g2": _nontrivial(n2g, 1.0), "b2": _nontrivial(n2b),
        "projb": _nontrivial(proj_b), "fc1b": _nontrivial(fc1_b),
        "fc2b": _nontrivial(fc2_b),
    }
    n_self = 4 if is_selfatt else 0
    nc = _get_nc(n_self, flags)

    bf = ml_dtypes.bfloat16
    projw_h = np.ascontiguousarray(
        np.asarray(proj_w, np.float32).reshape(CCH, P, C).transpose(1, 0, 2)
    ).astype(bf)
    fc1w_h = np.ascontiguousarray(
        np.asarray(fc1_w, np.float32).reshape(CCH, P, HID).transpose(1, 0, 2)
    ).astype(bf)
    fc2w_h = np.ascontiguousarray(np.asarray(fc2_w, np.float32)).astype(bf)
    id_h = np.eye(P, dtype=np.float32)
    selp_h = np.zeros((65, 2, P), np.float32)
    selp_h[64, 0, 0:64] = 1.0
    selp_h[64, 1, 64:128] = 1.0

    base = {
        "projw": projw_h, "fc1w": fc1w_h, "fc2w": fc2w_h,
        "id_bf": id_h.astype(bf), "selp": selp_h,
    }
    for nm, arr in (("g1", n1g), ("b1", n1b), ("g2", n2g), ("b2", n2b),
                    ("projb", proj_b), ("fc2b", fc2_b)):
        if flags[nm]:
            base[nm] = np.ascontiguousarray(np.asarray(arr, np.float32))
    if flags["fc1b"]:
        base["fc1b"] = np.ascontiguousarray(
            np.asarray(fc1_b, np.float32).reshape(HC, P).T)

    in_maps = []
    for core in range(N_CORES):
        bidx = core // 2
        own = x[bidx] if core % 2 == 0 else y[bidx]
        own_dev = np.ascontiguousarray(
            own.reshape(NT, P, C).transpose(1, 0, 2))
        m = dict(base)
        m["own"] = own_dev
        in_maps.append(m)

    res = run_bass_kernel_spmd(nc, in_maps, core_ids=list(range(N_CORES)))

    def unpack(core):
        o = np.asarray(res.results[core]["out"], np.float32)
        return o.transpose(1, 0, 2).reshape(N, C)

    x1 = np.stack([unpack(2 * b) for b in range(B)])
    y1 = np.stack([unpack(2 * b + 1) for b in range(B)])
    return (x1, y1)
